# revision 14
# baseline (speedup 1.0000x reference)
"""DbrxRouter MoE-routing kernel for 8 Trainium2 NeuronCores.

Computation (per reference):
  x = hidden_states.reshape(16384, 6144)
  logits = x @ W.T                      # [N, 16]
  weights = softmax(logits)             # fp32
  top_w, top_i = top_k(weights, 4)
  top_w = top_w / sum(top_w)
  returns (weights f32, top_w f32, top_i int32)

Sharding: token dim 16384 split 8 ways (2048 tokens/core); W replicated.

Numerics: x and W are split on the host into bf16 hi/lo pairs
(x = xh + xl, exact up to bf16(residual) rounding). logits are computed as
xh@Wh + xh@Wl + xl@Wh accumulated in fp32 PSUM. The dropped xl@Wl term is
~2^-16 relative; on the eval distribution this reproduces every top-4 index
(verified 0/65536 mismatches) with ~5e-6 weight error. The two bf16 halves
cost the same HBM bytes as the fp32 input.

Layout: the host pre-arranges each core's shard as [partition, chunk, tok]
so (a) the hidden/contraction dim lands on SBUF partitions with no on-chip
transposes and (b) every input DMA is a plain 2D block with 32 KiB
contiguous per partition (full DMA efficiency).

PE: only bf16 matmuls — stationary W-chunk [128h, 16e], moving x-chunk
[128h, 512tok]. Since M=16 uses 16/128 PE columns, the four 512-token
groups are packed into four PE column groups via tile_position=(0, 32g),
running concurrently; their logits.T accumulate at PSUM partitions 32g in
one bank. The last chunk is issued per-group so each group's softmax
overlaps the next group's matmuls. Softmax/top-k run batched per group:
PE transposes logits.T tiles to [128tok, 16]; DVE/ACT do max-subtract, exp,
normalize (free-dim-broadcast tensor_tensor), and InstMax8/InstMaxIndex for
the top-4.
"""

import sys

sys.path.insert(0, "/opt/trn_rl_repo")

import numpy as np
import ml_dtypes
import concourse.bass as bass
import concourse.mybir as mybir
from concourse import tile, masks
from concourse.bass_utils import run_bass_kernel_spmd
from contextlib import ExitStack

F32 = mybir.dt.float32
BF16 = mybir.dt.bfloat16
U32 = mybir.dt.uint32
I32 = mybir.dt.int32

N_CORES = 8
H = 6144
E = 16
P = 128
TOK_PER_CORE = 2048
NT = TOK_PER_CORE // P  # 16 token tiles per core
NCH = H // P  # 48 hidden chunks
NTG = 4  # token groups (PE column groups, 512 tokens each)
TG = TOK_PER_CORE // NTG  # 512
TPG = TG // P  # token tiles per group (4)

# DMA piece sizes in hidden chunks; small first pieces start matmuls early
PIECES = [2, 2, 2] + [3] * 14
MAXCH = max(PIECES)


def _split_sync_waits(nc, max_waits=1):
    """This walrus build accepts at most one sync wait per instruction
    ("Too many sync wait commands"). Move excess waits onto preceding
    NoOps on the same engine."""
    ctr = 0
    for fn in nc.m.functions:
        for bb in fn.blocks:
            new = []
            for inst in bb.instructions:
                si = inst.sync_info
                if (
                    si is not None
                    and si.on_wait
                    and len(si.on_wait) > max_waits
                    and inst.engine != mybir.EngineType.Unassigned
                ):
                    waits = list(si.on_wait)
                    for w in waits[max_waits:]:
                        ctr += 1
                        new.append(
                            mybir.InstNoOp(
                                name=f"I-ws{ctr}",
                                engine=inst.engine,
                                ins=[],
                                outs=[],
                                sync_info=mybir.SyncInfo(on_wait=[w], on_update=[]),
                            )
                        )
                    inst.sync_info = mybir.SyncInfo(
                        on_wait=waits[:max_waits],
                        on_update=list(si.on_update or []),
                    )
                new.append(inst)
            bb.instructions = new


def build():
    nc = bass.Bass("TRN2", target_bir_lowering=False, debug=False)
    # x halves: [p, c*TOK + tok] = x_half[h = c*128 + p, tok]
    xh_d = nc.dram_tensor("xh", [P, NCH * TOK_PER_CORE], BF16, kind="ExternalInput")
    xl_d = nc.dram_tensor("xl", [P, NCH * TOK_PER_CORE], BF16, kind="ExternalInput")
    # W halves: [p, c*E + e] = W_half[e, c*128 + p]
    wh_d = nc.dram_tensor("wh", [P, NCH * E], BF16, kind="ExternalInput")
    wl_d = nc.dram_tensor("wl", [P, NCH * E], BF16, kind="ExternalInput")
    # Outputs in on-chip layout; host reorders (see kernel()).
    wts_d = nc.dram_tensor("weights", [P, NT, E], F32, kind="ExternalOutput")
    topw_d = nc.dram_tensor("topw", [P, NT, 4], F32, kind="ExternalOutput")
    topi_d = nc.dram_tensor("topi", [P, NT, 8], I32, kind="ExternalOutput")

    with tile.TileContext(nc) as tc, ExitStack() as ctx:
        const = ctx.enter_context(tc.tile_pool(name="const", bufs=1))
        xpool = ctx.enter_context(tc.tile_pool(name="xp", bufs=8))
        ps_lgT = ctx.enter_context(tc.tile_pool(name="ps_lgT", bufs=1, space="PSUM"))
        ps_lg = ctx.enter_context(tc.tile_pool(name="ps_lg", bufs=2, space="PSUM"))
        small = ctx.enter_context(tc.tile_pool(name="small", bufs=4))

        # identity blocks at each 32-partition offset (for the K=16
        # logits.T transposes whose stationary sits at partition 32g)
        ident = const.tile([P, E], F32)
        for g in range(NTG):
            masks.make_identity(nc, ident[32 * g:32 * g + E, :])

        wh_sb = const.tile([P, NCH * E], BF16)
        nc.sync.dma_start(wh_sb[:], wh_d[:])
        wl_sb = const.tile([P, NCH * E], BF16)
        nc.scalar.dma_start(wl_sb[:], wl_d[:])

        # Output accumulators (DMAed out per token group).
        wts_acc = const.tile([P, NT * E], F32)
        topw_acc = const.tile([P, NT * 4], F32)
        topi_acc = const.tile([P, NT * 8], U32)

        # logits.T for all 4 groups in one PSUM bank: rows 32g..32g+16
        lgT = ps_lgT.tile([P, TG], F32, tag="lgT")

        def mm_xh(c, xh_c, a):
            """the two xh bf16 passes for chunk c, 4 column groups each"""
            w_s = slice(c * E, (c + 1) * E)
            for g in range(NTG):
                out = lgT[32 * g:32 * g + E, :]
                m = slice(a * TOK_PER_CORE + g * TG, a * TOK_PER_CORE + (g + 1) * TG)
                nc.tensor.matmul(
                    out, wh_sb[:, w_s], xh_c[:, m],
                    start=(c == 0), stop=False, tile_position=(0, 32 * g),
                )
                nc.tensor.matmul(
                    out, wl_sb[:, w_s], xh_c[:, m],
                    start=False, stop=False, tile_position=(0, 32 * g),
                )

        def mm_xl(c, xl_c, a, only_g=None):
            """the deferred xl pass; carries the accumulation stop flag"""
            w_s = slice(c * E, (c + 1) * E)
            for g in range(NTG) if only_g is None else [only_g]:
                out = lgT[32 * g:32 * g + E, :]
                m = slice(a * TOK_PER_CORE + g * TG, a * TOK_PER_CORE + (g + 1) * TG)
                nc.tensor.matmul(
                    out, wh_sb[:, w_s], xl_c[:, m],
                    start=False, stop=(c == NCH - 1), tile_position=(0, 32 * g),
                )

        def softmax_topk(g, lgT_sb):
            """batched softmax + top-4 for one 512-token group"""
            t0 = g * TPG
            # transpose the 4 [16,128] tiles -> [128,16] each, into one bank
            lg_ps = ps_lg.tile([P, TPG * E], F32, tag="lg")
            for tt in range(TPG):
                nc.tensor.transpose(
                    lg_ps[:, tt * E:(tt + 1) * E],
                    lgT_sb[32 * g:32 * g + E, tt * P:(tt + 1) * P],
                    ident[32 * g:32 * g + E, :],
                    tile_position=(32 * g, 0),
                )
            lg3 = lg_ps[:].rearrange("p (t e) -> p t e", e=E)
            mx = small.tile([P, TPG], F32, tag="mx")
            nc.vector.tensor_reduce(
                mx[:], lg3, axis=mybir.AxisListType.X, op=mybir.AluOpType.max
            )
            sh = small.tile([P, TPG * E], F32, tag="sh")
            nc.vector.tensor_tensor(
                sh[:].rearrange("p (t e) -> p t e", e=E),
                lg3,
                mx[:].broadcast_to([P, TPG, E]),
                mybir.AluOpType.subtract,
            )
            w_sl = wts_acc[:, t0 * E:(t0 + TPG) * E]
            nc.scalar.activation(
                w_sl, sh[:], mybir.ActivationFunctionType.Exp
            )
            ssum = small.tile([P, TPG], F32, tag="ssum")
            nc.vector.tensor_reduce(
                ssum[:],
                w_sl.rearrange("p (t e) -> p t e", e=E),
                axis=mybir.AxisListType.X,
                op=mybir.AluOpType.add,
            )
            rs = small.tile([P, TPG], F32, tag="rs")
            nc.vector.reciprocal(rs[:], ssum[:])
            nc.vector.tensor_tensor(
                w_sl.rearrange("p (t e) -> p t e", e=E),
                w_sl.rearrange("p (t e) -> p t e", e=E),
                rs[:].broadcast_to([P, TPG, E]),
                mybir.AluOpType.mult,
            )
            for tt in range(TPG):
                t = t0 + tt
                m8 = small.tile([P, 8], F32, tag="m8")
                w_t = wts_acc[:, t * E:(t + 1) * E]
                nc.vector.max(m8[:], w_t)
                nc.vector.max_index(topi_acc[:, t * 8:(t + 1) * 8], m8[:], w_t)
                nc.vector.tensor_copy(
                    topw_acc[:, t * 4:(t + 1) * 4], m8[:, :4]
                )
            tw = topw_acc[:, t0 * 4:(t0 + TPG) * 4]
            s4 = small.tile([P, TPG], F32, tag="s4")
            nc.vector.tensor_reduce(
                s4[:],
                tw.rearrange("p (t e) -> p t e", e=4),
                axis=mybir.AxisListType.X,
                op=mybir.AluOpType.add,
            )
            r4 = small.tile([P, TPG], F32, tag="r4")
            nc.vector.reciprocal(r4[:], s4[:])
            nc.vector.tensor_tensor(
                tw.rearrange("p (t e) -> p t e", e=4),
                tw.rearrange("p (t e) -> p t e", e=4),
                r4[:].broadcast_to([P, TPG, 4]),
                mybir.AluOpType.mult,
            )

        c0 = 0
        pending = []  # deferred xl passes: (chunk, tile, offset-in-tile)
        for piece in PIECES:
            fs = slice(c0 * TOK_PER_CORE, (c0 + piece) * TOK_PER_CORE)
            xh_c = xpool.tile([P, MAXCH * TOK_PER_CORE], BF16, tag="xh")
            nc.sync.dma_start(xh_c[:, :piece * TOK_PER_CORE], xh_d[:, fs])
            xl_c = xpool.tile([P, MAXCH * TOK_PER_CORE], BF16, tag="xl")
            nc.scalar.dma_start(xl_c[:, :piece * TOK_PER_CORE], xl_d[:, fs])
            for a in range(piece):
                mm_xh(c0 + a, xh_c, a)
            # xl passes of the previous piece (its DMA has had a full piece
            # of slack; the scalar ring lags the sync ring)
            for (c, t, a) in pending:
                mm_xl(c, t, a)
            pending = [(c0 + a, xl_c, a) for a in range(piece)]
            c0 += piece

        # tail: the final piece's xl passes, staggered per group so each
        # group's softmax overlaps the next group's matmuls
        lgT_sb = small.tile([P, TG], F32, tag="lgT_sb")
        for g in range(NTG):
            for (c, t, a) in pending:
                mm_xl(c, t, a, only_g=g)
            nc.vector.tensor_copy(
                lgT_sb[32 * g:32 * g + E, :], lgT[32 * g:32 * g + E, :]
            )
            softmax_topk(g, lgT_sb)

        nc.sync.dma_start(
            wts_d[:], wts_acc[:].rearrange("p (t e) -> p t e", e=E)
        )
        nc.scalar.dma_start(
            topw_d[:], topw_acc[:].rearrange("p (t e) -> p t e", e=4)
        )
        nc.gpsimd.dma_start(
            topi_d[:],
            topi_acc[:].bitcast(I32).rearrange("p (t e) -> p t e", e=8),
        )

    _split_sync_waits(nc)
    return nc


_NC_CACHE = []


def _prep_shards(hidden_states, W):
    bf16 = ml_dtypes.bfloat16
    x = hidden_states.reshape(-1, H).astype(np.float32, copy=False)
    w32 = W.astype(np.float32, copy=False)
    wh = w32.astype(bf16)
    wl = (w32 - wh.astype(np.float32)).astype(bf16)

    # [E, H] -> [P, NCH*E]: arr[p, c*E + e] = w[e, c*128 + p]
    def wlayout(a):
        return np.ascontiguousarray(
            a.T.reshape(NCH, P, E).transpose(1, 0, 2).reshape(P, NCH * E)
        )

    # [2048, 6144] half -> [P, NCH*TOK]: arr[p, c*TOK + t] = v[t, c*P + p]
    def xlayout(v):
        return np.ascontiguousarray(
            v.reshape(TOK_PER_CORE, NCH, P).transpose(2, 1, 0).reshape(P, -1)
        )

    wh_l, wl_l = wlayout(wh), wlayout(wl)
    in_maps = []
    for i in range(N_CORES):
        sh = x[i * TOK_PER_CORE:(i + 1) * TOK_PER_CORE]
        xh = sh.astype(bf16)
        xl = (sh - xh.astype(np.float32)).astype(bf16)
        in_maps.append(
            {"xh": xlayout(xh), "xl": xlayout(xl), "wh": wh_l, "wl": wl_l}
        )
    return in_maps


def kernel(hidden_states: np.ndarray, W: np.ndarray):
    n = hidden_states.reshape(-1, H).shape[0]
    assert n == N_CORES * TOK_PER_CORE

    if not _NC_CACHE:
        _NC_CACHE.append(build())
    nc = _NC_CACHE[0]

    in_maps = _prep_shards(hidden_states, W)
    res = run_bass_kernel_spmd(nc, in_maps, list(range(N_CORES)))

    weights = np.empty((n, E), np.float32)
    topw = np.empty((n, 4), np.float32)
    topi = np.empty((n, 4), np.int32)
    for i in range(N_CORES):
        r = res.results[i]
        s = slice(i * TOK_PER_CORE, (i + 1) * TOK_PER_CORE)
        # [P, NT, k] -> [NT, P, k] -> [2048, k]
        weights[s] = r["weights"].transpose(1, 0, 2).reshape(TOK_PER_CORE, E)
        topw[s] = r["topw"].transpose(1, 0, 2).reshape(TOK_PER_CORE, 4)
        topi[s] = r["topi"].transpose(1, 0, 2).reshape(TOK_PER_CORE, 8)[:, :4]
    dt = hidden_states.dtype
    return weights.astype(dt), topw.astype(dt), topi


# revision 15
# speedup vs baseline: 1.0223x; 1.0223x over previous
"""DbrxRouter MoE-routing kernel for 8 Trainium2 NeuronCores.

Computation (per reference):
  x = hidden_states.reshape(16384, 6144)
  logits = x @ W.T                      # [N, 16]
  weights = softmax(logits)             # fp32
  top_w, top_i = top_k(weights, 4)
  top_w = top_w / sum(top_w)
  returns (weights f32, top_w f32, top_i int32)

Sharding: token dim 16384 split 8 ways (2048 tokens/core); W replicated.

Numerics: x and W are split on the host into bf16 hi/lo pairs
(x = xh + xl, exact up to bf16(residual) rounding). logits are computed as
xh@Wh + xh@Wl + xl@Wh accumulated in fp32 PSUM. The dropped xl@Wl term is
~2^-16 relative; on the eval distribution this reproduces every top-4 index
(verified 0/65536 mismatches) with ~5e-6 weight error. The two bf16 halves
cost the same HBM bytes as the fp32 input.

Layout: the host pre-arranges each core's shard as [partition, chunk, tok]
so (a) the hidden/contraction dim lands on SBUF partitions with no on-chip
transposes and (b) every input DMA is a plain 2D block with 32 KiB
contiguous per partition (full DMA efficiency).

PE: only bf16 matmuls — stationary W-chunk [128h, 16e], moving x-chunk
[128h, 512tok]. Since M=16 uses 16/128 PE columns, the four 512-token
groups are packed into four PE column groups via tile_position=(0, 32g),
running concurrently; their logits.T accumulate at PSUM partitions 32g in
one bank. The last chunk is issued per-group so each group's softmax
overlaps the next group's matmuls. Softmax/top-k run batched per group:
PE transposes logits.T tiles to [128tok, 16]; DVE/ACT do max-subtract, exp,
normalize (free-dim-broadcast tensor_tensor), and InstMax8/InstMaxIndex for
the top-4.
"""

import sys

sys.path.insert(0, "/opt/trn_rl_repo")

import numpy as np
import ml_dtypes
import concourse.bass as bass
import concourse.mybir as mybir
from concourse import tile, masks
from concourse.bass_utils import run_bass_kernel_spmd
from contextlib import ExitStack

F32 = mybir.dt.float32
BF16 = mybir.dt.bfloat16
U32 = mybir.dt.uint32
I32 = mybir.dt.int32

N_CORES = 8
H = 6144
E = 16
P = 128
TOK_PER_CORE = 2048
NT = TOK_PER_CORE // P  # 16 token tiles per core
NCH = H // P  # 48 hidden chunks
NTG = 4  # token groups (PE column groups, 512 tokens each)
TG = TOK_PER_CORE // NTG  # 512
TPG = TG // P  # token tiles per group (4)

# DMA piece sizes in hidden chunks; small first pieces start matmuls early
PIECES = [2, 2] + [4] * 11
MAXCH = max(PIECES)


def _split_sync_waits(nc, max_waits=1):
    """This walrus build accepts at most one sync wait per instruction
    ("Too many sync wait commands"). Move excess waits onto preceding
    NoOps on the same engine."""
    ctr = 0
    for fn in nc.m.functions:
        for bb in fn.blocks:
            new = []
            for inst in bb.instructions:
                si = inst.sync_info
                if (
                    si is not None
                    and si.on_wait
                    and len(si.on_wait) > max_waits
                    and inst.engine != mybir.EngineType.Unassigned
                ):
                    waits = list(si.on_wait)
                    for w in waits[max_waits:]:
                        ctr += 1
                        new.append(
                            mybir.InstNoOp(
                                name=f"I-ws{ctr}",
                                engine=inst.engine,
                                ins=[],
                                outs=[],
                                sync_info=mybir.SyncInfo(on_wait=[w], on_update=[]),
                            )
                        )
                    inst.sync_info = mybir.SyncInfo(
                        on_wait=waits[:max_waits],
                        on_update=list(si.on_update or []),
                    )
                new.append(inst)
            bb.instructions = new


def build():
    nc = bass.Bass("TRN2", target_bir_lowering=False, debug=False)
    # x halves: [p, c*TOK + tok] = x_half[h = c*128 + p, tok]
    xh_d = nc.dram_tensor("xh", [P, NCH * TOK_PER_CORE], BF16, kind="ExternalInput")
    xl_d = nc.dram_tensor("xl", [P, NCH * TOK_PER_CORE], BF16, kind="ExternalInput")
    # W halves: [p, c*E + e] = W_half[e, c*128 + p]
    wh_d = nc.dram_tensor("wh", [P, NCH * E], BF16, kind="ExternalInput")
    wl_d = nc.dram_tensor("wl", [P, NCH * E], BF16, kind="ExternalInput")
    # Outputs in on-chip layout; host reorders (see kernel()).
    wts_d = nc.dram_tensor("weights", [P, NT, E], F32, kind="ExternalOutput")
    topw_d = nc.dram_tensor("topw", [P, NT, 4], F32, kind="ExternalOutput")
    topi_d = nc.dram_tensor("topi", [P, NT, 8], I32, kind="ExternalOutput")

    with tile.TileContext(nc) as tc, ExitStack() as ctx:
        const = ctx.enter_context(tc.tile_pool(name="const", bufs=1))
        xpool = ctx.enter_context(tc.tile_pool(name="xp", bufs=6))
        ps_lgT = ctx.enter_context(tc.tile_pool(name="ps_lgT", bufs=1, space="PSUM"))
        ps_lg = ctx.enter_context(tc.tile_pool(name="ps_lg", bufs=2, space="PSUM"))
        small = ctx.enter_context(tc.tile_pool(name="small", bufs=4))

        # identity blocks at each 32-partition offset (for the K=16
        # logits.T transposes whose stationary sits at partition 32g)
        ident = const.tile([P, E], F32)
        for g in range(NTG):
            masks.make_identity(nc, ident[32 * g:32 * g + E, :])

        wh_sb = const.tile([P, NCH * E], BF16)
        nc.sync.dma_start(wh_sb[:], wh_d[:])
        wl_sb = const.tile([P, NCH * E], BF16)
        nc.scalar.dma_start(wl_sb[:], wl_d[:])

        # Output accumulators (DMAed out per token group).
        wts_acc = const.tile([P, NT * E], F32)
        topw_acc = const.tile([P, NT * 4], F32)
        topi_acc = const.tile([P, NT * 8], U32)

        # logits.T for all 4 groups in one PSUM bank: rows 32g..32g+16
        lgT = ps_lgT.tile([P, TG], F32, tag="lgT")

        def mm_xh(c, xh_c, a):
            """the two xh bf16 passes for chunk c, 4 column groups each"""
            w_s = slice(c * E, (c + 1) * E)
            for g in range(NTG):
                out = lgT[32 * g:32 * g + E, :]
                m = slice(a * TOK_PER_CORE + g * TG, a * TOK_PER_CORE + (g + 1) * TG)
                nc.tensor.matmul(
                    out, wh_sb[:, w_s], xh_c[:, m],
                    start=(c == 0), stop=False, tile_position=(0, 32 * g),
                )
                nc.tensor.matmul(
                    out, wl_sb[:, w_s], xh_c[:, m],
                    start=False, stop=False, tile_position=(0, 32 * g),
                )

        def mm_xl(c, xl_c, a, only_g=None):
            """the deferred xl pass; carries the accumulation stop flag"""
            w_s = slice(c * E, (c + 1) * E)
            for g in range(NTG) if only_g is None else [only_g]:
                out = lgT[32 * g:32 * g + E, :]
                m = slice(a * TOK_PER_CORE + g * TG, a * TOK_PER_CORE + (g + 1) * TG)
                nc.tensor.matmul(
                    out, wh_sb[:, w_s], xl_c[:, m],
                    start=False, stop=(c == NCH - 1), tile_position=(0, 32 * g),
                )

        def softmax_topk(g, lgT_sb):
            """batched softmax + top-4 for one 512-token group"""
            t0 = g * TPG
            # transpose the 4 [16,128] tiles -> [128,16] each, into one bank
            lg_ps = ps_lg.tile([P, TPG * E], F32, tag="lg")
            for tt in range(TPG):
                nc.tensor.transpose(
                    lg_ps[:, tt * E:(tt + 1) * E],
                    lgT_sb[32 * g:32 * g + E, tt * P:(tt + 1) * P],
                    ident[32 * g:32 * g + E, :],
                    tile_position=(32 * g, 0),
                )
            lg3 = lg_ps[:].rearrange("p (t e) -> p t e", e=E)
            mx = small.tile([P, TPG], F32, tag="mx")
            nc.vector.tensor_reduce(
                mx[:], lg3, axis=mybir.AxisListType.X, op=mybir.AluOpType.max
            )
            sh = small.tile([P, TPG * E], F32, tag="sh")
            nc.vector.tensor_tensor(
                sh[:].rearrange("p (t e) -> p t e", e=E),
                lg3,
                mx[:].broadcast_to([P, TPG, E]),
                mybir.AluOpType.subtract,
            )
            w_sl = wts_acc[:, t0 * E:(t0 + TPG) * E]
            nc.scalar.activation(
                w_sl, sh[:], mybir.ActivationFunctionType.Exp
            )
            ssum = small.tile([P, TPG], F32, tag="ssum")
            nc.vector.tensor_reduce(
                ssum[:],
                w_sl.rearrange("p (t e) -> p t e", e=E),
                axis=mybir.AxisListType.X,
                op=mybir.AluOpType.add,
            )
            rs = small.tile([P, TPG], F32, tag="rs")
            nc.vector.reciprocal(rs[:], ssum[:])
            nc.vector.tensor_tensor(
                w_sl.rearrange("p (t e) -> p t e", e=E),
                w_sl.rearrange("p (t e) -> p t e", e=E),
                rs[:].broadcast_to([P, TPG, E]),
                mybir.AluOpType.mult,
            )
            for tt in range(TPG):
                t = t0 + tt
                m8 = small.tile([P, 8], F32, tag="m8")
                w_t = wts_acc[:, t * E:(t + 1) * E]
                nc.vector.max(m8[:], w_t)
                nc.vector.max_index(topi_acc[:, t * 8:(t + 1) * 8], m8[:], w_t)
                nc.vector.tensor_copy(
                    topw_acc[:, t * 4:(t + 1) * 4], m8[:, :4]
                )
            tw = topw_acc[:, t0 * 4:(t0 + TPG) * 4]
            s4 = small.tile([P, TPG], F32, tag="s4")
            nc.vector.tensor_reduce(
                s4[:],
                tw.rearrange("p (t e) -> p t e", e=4),
                axis=mybir.AxisListType.X,
                op=mybir.AluOpType.add,
            )
            r4 = small.tile([P, TPG], F32, tag="r4")
            nc.vector.reciprocal(r4[:], s4[:])
            nc.vector.tensor_tensor(
                tw.rearrange("p (t e) -> p t e", e=4),
                tw.rearrange("p (t e) -> p t e", e=4),
                r4[:].broadcast_to([P, TPG, 4]),
                mybir.AluOpType.mult,
            )

        c0 = 0
        pending = []  # deferred xl passes: (chunk, tile, offset-in-tile)
        for piece in PIECES:
            fs = slice(c0 * TOK_PER_CORE, (c0 + piece) * TOK_PER_CORE)
            xh_c = xpool.tile([P, MAXCH * TOK_PER_CORE], BF16, tag="xh")
            nc.sync.dma_start(xh_c[:, :piece * TOK_PER_CORE], xh_d[:, fs])
            xl_c = xpool.tile([P, MAXCH * TOK_PER_CORE], BF16, tag="xl")
            nc.sync.dma_start(xl_c[:, :piece * TOK_PER_CORE], xl_d[:, fs])
            for a in range(piece):
                mm_xh(c0 + a, xh_c, a)
            # xl passes of the previous piece (its DMA has had a full piece
            # of slack; the scalar ring lags the sync ring)
            for (c, t, a) in pending:
                mm_xl(c, t, a)
            pending = [(c0 + a, xl_c, a) for a in range(piece)]
            c0 += piece

        # tail: the final piece's xl passes, staggered per group so each
        # group's softmax overlaps the next group's matmuls
        lgT_sb = small.tile([P, TG], F32, tag="lgT_sb")
        for g in range(NTG):
            for (c, t, a) in pending:
                mm_xl(c, t, a, only_g=g)
            nc.vector.tensor_copy(
                lgT_sb[32 * g:32 * g + E, :], lgT[32 * g:32 * g + E, :]
            )
            softmax_topk(g, lgT_sb)

        nc.sync.dma_start(
            wts_d[:], wts_acc[:].rearrange("p (t e) -> p t e", e=E)
        )
        nc.scalar.dma_start(
            topw_d[:], topw_acc[:].rearrange("p (t e) -> p t e", e=4)
        )
        nc.gpsimd.dma_start(
            topi_d[:],
            topi_acc[:].bitcast(I32).rearrange("p (t e) -> p t e", e=8),
        )

    _split_sync_waits(nc)
    return nc


_NC_CACHE = []


def _prep_shards(hidden_states, W):
    bf16 = ml_dtypes.bfloat16
    x = hidden_states.reshape(-1, H).astype(np.float32, copy=False)
    w32 = W.astype(np.float32, copy=False)
    wh = w32.astype(bf16)
    wl = (w32 - wh.astype(np.float32)).astype(bf16)

    # [E, H] -> [P, NCH*E]: arr[p, c*E + e] = w[e, c*128 + p]
    def wlayout(a):
        return np.ascontiguousarray(
            a.T.reshape(NCH, P, E).transpose(1, 0, 2).reshape(P, NCH * E)
        )

    # [2048, 6144] half -> [P, NCH*TOK]: arr[p, c*TOK + t] = v[t, c*P + p]
    def xlayout(v):
        return np.ascontiguousarray(
            v.reshape(TOK_PER_CORE, NCH, P).transpose(2, 1, 0).reshape(P, -1)
        )

    wh_l, wl_l = wlayout(wh), wlayout(wl)
    in_maps = []
    for i in range(N_CORES):
        sh = x[i * TOK_PER_CORE:(i + 1) * TOK_PER_CORE]
        xh = sh.astype(bf16)
        xl = (sh - xh.astype(np.float32)).astype(bf16)
        in_maps.append(
            {"xh": xlayout(xh), "xl": xlayout(xl), "wh": wh_l, "wl": wl_l}
        )
    return in_maps


def kernel(hidden_states: np.ndarray, W: np.ndarray):
    n = hidden_states.reshape(-1, H).shape[0]
    assert n == N_CORES * TOK_PER_CORE

    if not _NC_CACHE:
        _NC_CACHE.append(build())
    nc = _NC_CACHE[0]

    in_maps = _prep_shards(hidden_states, W)
    res = run_bass_kernel_spmd(nc, in_maps, list(range(N_CORES)))

    weights = np.empty((n, E), np.float32)
    topw = np.empty((n, 4), np.float32)
    topi = np.empty((n, 4), np.int32)
    for i in range(N_CORES):
        r = res.results[i]
        s = slice(i * TOK_PER_CORE, (i + 1) * TOK_PER_CORE)
        # [P, NT, k] -> [NT, P, k] -> [2048, k]
        weights[s] = r["weights"].transpose(1, 0, 2).reshape(TOK_PER_CORE, E)
        topw[s] = r["topw"].transpose(1, 0, 2).reshape(TOK_PER_CORE, 4)
        topi[s] = r["topi"].transpose(1, 0, 2).reshape(TOK_PER_CORE, 8)[:, :4]
    dt = hidden_states.dtype
    return weights.astype(dt), topw.astype(dt), topi


# revision 16
# speedup vs baseline: 1.0916x; 1.0677x over previous
"""DbrxRouter MoE-routing kernel for 8 Trainium2 NeuronCores.

Computation (per reference):
  x = hidden_states.reshape(16384, 6144)
  logits = x @ W.T                      # [N, 16]
  weights = softmax(logits)             # fp32
  top_w, top_i = top_k(weights, 4)
  top_w = top_w / sum(top_w)
  returns (weights f32, top_w f32, top_i int32)

Sharding: token dim 16384 split 8 ways (2048 tokens/core); W replicated.

Numerics: x and W are split on the host into bf16 hi/lo pairs
(x = xh + xl, exact up to bf16(residual) rounding). logits are computed as
xh@Wh + xh@Wl + xl@Wh accumulated in fp32 PSUM. The dropped xl@Wl term is
~2^-16 relative; on the eval distribution this reproduces every top-4 index
(verified 0/65536 mismatches) with ~5e-6 weight error. The two bf16 halves
cost the same HBM bytes as the fp32 input.

Layout: the host pre-arranges each core's shard as [partition, chunk, tok]
so (a) the hidden/contraction dim lands on SBUF partitions with no on-chip
transposes and (b) every input DMA is a plain 2D block with 32 KiB
contiguous per partition (full DMA efficiency).

PE: only bf16 matmuls — stationary W-chunk [128h, 16e], moving x-chunk
[128h, 512tok]. Since M=16 uses 16/128 PE columns, the four 512-token
groups are packed into four PE column groups via tile_position=(0, 32g),
running concurrently; their logits.T accumulate at PSUM partitions 32g in
one bank. The last chunk is issued per-group so each group's softmax
overlaps the next group's matmuls. Softmax/top-k run batched per group:
PE transposes logits.T tiles to [128tok, 16]; DVE/ACT do max-subtract, exp,
normalize (free-dim-broadcast tensor_tensor), and InstMax8/InstMaxIndex for
the top-4.
"""

import sys

sys.path.insert(0, "/opt/trn_rl_repo")

import numpy as np
import ml_dtypes
import concourse.bass as bass
import concourse.mybir as mybir
from concourse import tile, masks
from concourse.bass_utils import run_bass_kernel_spmd
from contextlib import ExitStack

F32 = mybir.dt.float32
BF16 = mybir.dt.bfloat16
U32 = mybir.dt.uint32
I32 = mybir.dt.int32

N_CORES = 8
H = 6144
E = 16
P = 128
TOK_PER_CORE = 2048
NT = TOK_PER_CORE // P  # 16 token tiles per core
NCH = H // P  # 48 hidden chunks
NTG = 4  # token groups (PE column groups, 512 tokens each)
TG = TOK_PER_CORE // NTG  # 512
TPG = TG // P  # token tiles per group (4)

# DMA piece sizes in hidden chunks; small first pieces start matmuls early
PIECES = [2, 2] + [4] * 11
MAXCH = max(PIECES)


def _split_sync_waits(nc, max_waits=1):
    """This walrus build accepts at most one sync wait per instruction
    ("Too many sync wait commands"). Move excess waits onto preceding
    NoOps on the same engine."""
    ctr = 0
    for fn in nc.m.functions:
        for bb in fn.blocks:
            new = []
            for inst in bb.instructions:
                si = inst.sync_info
                if (
                    si is not None
                    and si.on_wait
                    and len(si.on_wait) > max_waits
                    and inst.engine != mybir.EngineType.Unassigned
                ):
                    waits = list(si.on_wait)
                    for w in waits[max_waits:]:
                        ctr += 1
                        new.append(
                            mybir.InstNoOp(
                                name=f"I-ws{ctr}",
                                engine=inst.engine,
                                ins=[],
                                outs=[],
                                sync_info=mybir.SyncInfo(on_wait=[w], on_update=[]),
                            )
                        )
                    inst.sync_info = mybir.SyncInfo(
                        on_wait=waits[:max_waits],
                        on_update=list(si.on_update or []),
                    )
                new.append(inst)
            bb.instructions = new


def build():
    nc = bass.Bass("TRN2", target_bir_lowering=False, debug=False)
    # x halves: [p, c*TOK + tok] = x_half[h = c*128 + p, tok]
    xh_d = nc.dram_tensor("xh", [P, NCH * TOK_PER_CORE], BF16, kind="ExternalInput")
    xl_d = nc.dram_tensor("xl", [P, NCH * TOK_PER_CORE], BF16, kind="ExternalInput")
    # W halves: [p, c*E + e] = W_half[e, c*128 + p]
    wh_d = nc.dram_tensor("wh", [P, NCH * E], BF16, kind="ExternalInput")
    wl_d = nc.dram_tensor("wl", [P, NCH * E], BF16, kind="ExternalInput")
    # Outputs in on-chip layout; host reorders (see kernel()).
    wts_d = nc.dram_tensor("weights", [P, NT, E], F32, kind="ExternalOutput")
    topw_d = nc.dram_tensor("topw", [P, NT, 4], F32, kind="ExternalOutput")
    topi_d = nc.dram_tensor("topi", [P, NT, 8], I32, kind="ExternalOutput")

    with tile.TileContext(nc) as tc, ExitStack() as ctx:
        const = ctx.enter_context(tc.tile_pool(name="const", bufs=1))
        xpool = ctx.enter_context(tc.tile_pool(name="xp", bufs=6))
        ps_lgT = ctx.enter_context(tc.tile_pool(name="ps_lgT", bufs=1, space="PSUM"))
        ps_lg = ctx.enter_context(tc.tile_pool(name="ps_lg", bufs=2, space="PSUM"))
        small = ctx.enter_context(tc.tile_pool(name="small", bufs=4))

        # identity blocks at each 32-partition offset (for the K=16
        # logits.T transposes whose stationary sits at partition 32g)
        ident = const.tile([P, E], F32)
        for g in range(NTG):
            masks.make_identity(nc, ident[32 * g:32 * g + E, :])

        wh_sb = const.tile([P, NCH * E], BF16)
        nc.sync.dma_start(wh_sb[:], wh_d[:])
        wl_sb = const.tile([P, NCH * E], BF16)
        nc.scalar.dma_start(wl_sb[:], wl_d[:])

        # Output accumulators (DMAed out per token group).
        wts_acc = const.tile([P, NT * E], F32)
        topw_acc = const.tile([P, NT * 4], F32)
        topi_acc = const.tile([P, NT * 8], U32)

        # logits.T for all 4 groups in one PSUM bank: rows 32g..32g+16
        lgT = ps_lgT.tile([P, TG], F32, tag="lgT")

        def mm_xh(c, xh_c, a):
            """the two xh bf16 passes for chunk c, 4 column groups each"""
            w_s = slice(c * E, (c + 1) * E)
            for g in range(NTG):
                out = lgT[32 * g:32 * g + E, :]
                m = slice(a * TOK_PER_CORE + g * TG, a * TOK_PER_CORE + (g + 1) * TG)
                nc.tensor.matmul(
                    out, wh_sb[:, w_s], xh_c[:, m],
                    start=(c == 0), stop=False, tile_position=(0, 32 * g),
                )
                nc.tensor.matmul(
                    out, wl_sb[:, w_s], xh_c[:, m],
                    start=False, stop=False, tile_position=(0, 32 * g),
                )

        def mm_xl(c, xl_c, a, only_g=None):
            """the deferred xl pass; carries the accumulation stop flag"""
            w_s = slice(c * E, (c + 1) * E)
            for g in range(NTG) if only_g is None else [only_g]:
                out = lgT[32 * g:32 * g + E, :]
                m = slice(a * TOK_PER_CORE + g * TG, a * TOK_PER_CORE + (g + 1) * TG)
                nc.tensor.matmul(
                    out, wh_sb[:, w_s], xl_c[:, m],
                    start=False, stop=(c == NCH - 1), tile_position=(0, 32 * g),
                )

        def softmax_topk(g, lgT_sb):
            """batched softmax + top-4 for one 512-token group"""
            t0 = g * TPG
            # transpose the 4 [16,128] tiles -> [128,16] each, into one bank
            lg_ps = ps_lg.tile([P, TPG * E], F32, tag="lg")
            for tt in range(TPG):
                nc.tensor.transpose(
                    lg_ps[:, tt * E:(tt + 1) * E],
                    lgT_sb[32 * g:32 * g + E, tt * P:(tt + 1) * P],
                    ident[32 * g:32 * g + E, :],
                    tile_position=(32 * g, 0),
                )
            lg3 = lg_ps[:].rearrange("p (t e) -> p t e", e=E)
            mx = small.tile([P, TPG], F32, tag="mx")
            nc.vector.tensor_reduce(
                mx[:], lg3, axis=mybir.AxisListType.X, op=mybir.AluOpType.max
            )
            sh = small.tile([P, TPG * E], F32, tag="sh")
            nc.vector.tensor_tensor(
                sh[:].rearrange("p (t e) -> p t e", e=E),
                lg3,
                mx[:].broadcast_to([P, TPG, E]),
                mybir.AluOpType.subtract,
            )
            w_sl = wts_acc[:, t0 * E:(t0 + TPG) * E]
            nc.scalar.activation(
                w_sl, sh[:], mybir.ActivationFunctionType.Exp
            )
            ssum = small.tile([P, TPG], F32, tag="ssum")
            nc.vector.tensor_reduce(
                ssum[:],
                w_sl.rearrange("p (t e) -> p t e", e=E),
                axis=mybir.AxisListType.X,
                op=mybir.AluOpType.add,
            )
            rs = small.tile([P, TPG], F32, tag="rs")
            nc.vector.reciprocal(rs[:], ssum[:])
            nc.vector.tensor_tensor(
                w_sl.rearrange("p (t e) -> p t e", e=E),
                w_sl.rearrange("p (t e) -> p t e", e=E),
                rs[:].broadcast_to([P, TPG, E]),
                mybir.AluOpType.mult,
            )
            for tt in range(TPG):
                t = t0 + tt
                m8 = small.tile([P, 8], F32, tag="m8")
                w_t = wts_acc[:, t * E:(t + 1) * E]
                nc.vector.max(m8[:], w_t)
                nc.vector.max_index(topi_acc[:, t * 8:(t + 1) * 8], m8[:], w_t)
                nc.vector.tensor_copy(
                    topw_acc[:, t * 4:(t + 1) * 4], m8[:, :4]
                )
            tw = topw_acc[:, t0 * 4:(t0 + TPG) * 4]
            s4 = small.tile([P, TPG], F32, tag="s4")
            nc.vector.tensor_reduce(
                s4[:],
                tw.rearrange("p (t e) -> p t e", e=4),
                axis=mybir.AxisListType.X,
                op=mybir.AluOpType.add,
            )
            r4 = small.tile([P, TPG], F32, tag="r4")
            nc.vector.reciprocal(r4[:], s4[:])
            nc.vector.tensor_tensor(
                tw.rearrange("p (t e) -> p t e", e=4),
                tw.rearrange("p (t e) -> p t e", e=4),
                r4[:].broadcast_to([P, TPG, 4]),
                mybir.AluOpType.mult,
            )

        c0 = 0
        pending = []  # deferred xl passes: (chunk, tile, offset-in-tile)
        for piece in PIECES:
            fs = slice(c0 * TOK_PER_CORE, (c0 + piece) * TOK_PER_CORE)
            xh_c = xpool.tile([P, MAXCH * TOK_PER_CORE], BF16, tag="xh")
            nc.sync.dma_start(xh_c[:, :piece * TOK_PER_CORE], xh_d[:, fs])
            xl_c = xpool.tile([P, MAXCH * TOK_PER_CORE], BF16, tag="xl")
            nc.scalar.dma_start(xl_c[:, :piece * TOK_PER_CORE], xl_d[:, fs])
            for a in range(piece):
                mm_xh(c0 + a, xh_c, a)
            # xl passes of the previous piece (its DMA has had a full piece
            # of slack; the scalar ring lags the sync ring)
            for (c, t, a) in pending:
                mm_xl(c, t, a)
            pending = [(c0 + a, xl_c, a) for a in range(piece)]
            c0 += piece

        # tail: the final piece's xl passes, staggered per group so each
        # group's softmax overlaps the next group's matmuls
        lgT_sb = small.tile([P, TG], F32, tag="lgT_sb")
        for g in range(NTG):
            for (c, t, a) in pending:
                mm_xl(c, t, a, only_g=g)
            nc.vector.tensor_copy(
                lgT_sb[32 * g:32 * g + E, :], lgT[32 * g:32 * g + E, :]
            )
            softmax_topk(g, lgT_sb)

        nc.sync.dma_start(
            wts_d[:], wts_acc[:].rearrange("p (t e) -> p t e", e=E)
        )
        nc.scalar.dma_start(
            topw_d[:], topw_acc[:].rearrange("p (t e) -> p t e", e=4)
        )
        nc.gpsimd.dma_start(
            topi_d[:],
            topi_acc[:].bitcast(I32).rearrange("p (t e) -> p t e", e=8),
        )

    _split_sync_waits(nc)
    return nc


_NC_CACHE = []


def _prep_shards(hidden_states, W):
    bf16 = ml_dtypes.bfloat16
    x = hidden_states.reshape(-1, H).astype(np.float32, copy=False)
    w32 = W.astype(np.float32, copy=False)
    wh = w32.astype(bf16)
    wl = (w32 - wh.astype(np.float32)).astype(bf16)

    # [E, H] -> [P, NCH*E]: arr[p, c*E + e] = w[e, c*128 + p]
    def wlayout(a):
        return np.ascontiguousarray(
            a.T.reshape(NCH, P, E).transpose(1, 0, 2).reshape(P, NCH * E)
        )

    # [2048, 6144] half -> [P, NCH*TOK]: arr[p, c*TOK + t] = v[t, c*P + p]
    def xlayout(v):
        return np.ascontiguousarray(
            v.reshape(TOK_PER_CORE, NCH, P).transpose(2, 1, 0).reshape(P, -1)
        )

    wh_l, wl_l = wlayout(wh), wlayout(wl)
    in_maps = []
    for i in range(N_CORES):
        sh = x[i * TOK_PER_CORE:(i + 1) * TOK_PER_CORE]
        xh = sh.astype(bf16)
        xl = (sh - xh.astype(np.float32)).astype(bf16)
        in_maps.append(
            {"xh": xlayout(xh), "xl": xlayout(xl), "wh": wh_l, "wl": wl_l}
        )
    return in_maps


def kernel(hidden_states: np.ndarray, W: np.ndarray):
    n = hidden_states.reshape(-1, H).shape[0]
    assert n == N_CORES * TOK_PER_CORE

    if not _NC_CACHE:
        _NC_CACHE.append(build())
    nc = _NC_CACHE[0]

    in_maps = _prep_shards(hidden_states, W)
    res = run_bass_kernel_spmd(nc, in_maps, list(range(N_CORES)))

    weights = np.empty((n, E), np.float32)
    topw = np.empty((n, 4), np.float32)
    topi = np.empty((n, 4), np.int32)
    for i in range(N_CORES):
        r = res.results[i]
        s = slice(i * TOK_PER_CORE, (i + 1) * TOK_PER_CORE)
        # [P, NT, k] -> [NT, P, k] -> [2048, k]
        weights[s] = r["weights"].transpose(1, 0, 2).reshape(TOK_PER_CORE, E)
        topw[s] = r["topw"].transpose(1, 0, 2).reshape(TOK_PER_CORE, 4)
        topi[s] = r["topi"].transpose(1, 0, 2).reshape(TOK_PER_CORE, 8)[:, :4]
    dt = hidden_states.dtype
    return weights.astype(dt), topw.astype(dt), topi
